# revision 47
# baseline (speedup 1.0000x reference)
"""BartLatentAttention Trainium2 kernel.

Full-input contract: kernel(**inputs) takes the unsharded tensors from
setup_inputs() and returns the full [B, T, D] float32 output.

Sharding: tensor-parallel over heads. 16 heads / 8 cores = 2 heads per
core. Each core computes q/k/v projections for its 2 heads (column-sliced
weights), attention over the latent-prefixed sequence, and a partial
output projection (row-sliced Wo). The host sums the 8 partial outputs
and adds bo.

Device-side layout notes:
  - hidden is fed pre-transposed AND pre-chunked as hpre [8, 128, 8, 512]
    bf16 (g-chunk, partition, k-chunk, token) so each g-chunk's load is a
    single DMA with one contiguous 8KB run per partition (128 descriptors
    instead of 1024 -> ~6x faster descriptor generation, which was the
    phase-1 critical path). Weights likewise land as [128, 8*128] blocks.
  - scores are computed transposed (scoresT [s, t]) so softmax's exp runs
    on ScalarE straight out of PSUM, and the AV matmul consumes expT as
    the moving operand with V [s, d] stationary. The two heads' score
    matmuls land on PE row-tiles (0,0)/(64,0) and execute concurrently.
  - V carries an extra ones-column (M=65): PSUM row 64 of the AV
    accumulation is the softmax denominator Z for free.
  - S = L + T = 2056 is laid out padded to 2176 = 17*128: chunk 0 holds
    the 8 latent positions + 120 dead rows (killed with an exp bias of
    -30), chunks 1..16 hold the 2048 token positions.
  - ScalarE exp is the bottleneck engine (136 instrs x ~1.07us = 146us
    busy); phase 2 paces side work (batch-1 qkv, epilogues) at ~1.6
    matmul-equivalents per attention chunk so exp never starves.
  - side work drains as CHAINS: the "mm"-tag PSUM ring (bufs=2) rotates
    in trace order, so a projection's accumulation chain must not have
    other mm-tag allocations traced in its middle. Chunk bodies (sc/av/ex
    tags) may interleave anywhere; epilogue chains only run between qkv
    chains.
  - epilogue is all-bf16 (zb broadcast matmul, ot, wo); out-projection
    tiles are cast to fp16 into one [128, 1024] tile per 128 rows (one
    DMA each, alternating sync/gpsimd rings) and the host sums the 8
    partials in fp64.
  - ~36 junk 128-col matmuls at t=0 (on a memset tile, so they don't wait
    for the gpsimd-made identity) trip the PE HAM activity monitor so the
    real work starts at 2.4 GHz.
"""

import sys

if "/opt/trn_rl_repo" not in sys.path:
    sys.path.insert(0, "/opt/trn_rl_repo")

import numpy as np
import ml_dtypes

BF16 = ml_dtypes.bfloat16

B, T, D = 2, 2048, 1024
H = 16
HD = D // H  # 64
L = 8
S = L + T  # 2056
SCALE = HD ** -0.5
NCORES = 8
HPC = H // NCORES  # heads per core = 2
DC = HPC * HD  # per-core feature width = 128

BT = B * T  # 4096
NKC = D // 128  # k chunks = 8
NTC = BT // 512  # token chunks of 512 = 8
SCHUNKS = 17  # padded S = 2176 = 17 * 128
TB = 512  # attention t-block
PAD_BIAS = -30.0

_cache: dict = {}


def _build_nc():
    import concourse.bass as bass
    import concourse.mybir as mybir
    import concourse.tile as tile
    from concourse import bacc

    fp32 = mybir.dt.float32
    bf16 = mybir.dt.bfloat16
    fp16 = mybir.dt.float16

    nc = bacc.Bacc(
        "TRN2",
        target_bir_lowering=False,
        debug=False,
        enable_asserts=False,
        num_devices=NCORES,
    )

    # DRAM I/O (all host-pre-arranged for contiguous per-partition runs)
    hpre = nc.dram_tensor("hpre", [NTC, 128, NKC, 512], bf16,
                          kind="ExternalInput").ap()
    wq = nc.dram_tensor("wq", [128, NKC * DC], bf16, kind="ExternalInput").ap()
    wk = nc.dram_tensor("wk", [128, NKC * DC], bf16, kind="ExternalInput").ap()
    wv = nc.dram_tensor("wv", [128, NKC * DC], bf16, kind="ExternalInput").ap()
    bq = nc.dram_tensor("bq", [DC, 1], fp32, kind="ExternalInput").ap()
    bk = nc.dram_tensor("bk", [DC, 1], fp32, kind="ExternalInput").ap()
    bv1 = nc.dram_tensor("bv1", [DC, 1], fp32, kind="ExternalInput").ap()
    ebias0 = nc.dram_tensor("ebias0", [128, 1], fp32, kind="ExternalInput").ap()
    wo = nc.dram_tensor("wo", [DC, D], bf16, kind="ExternalInput").ap()
    lkT = nc.dram_tensor("lkT", [B, DC, L], bf16, kind="ExternalInput").ap()
    lv = nc.dram_tensor("lv", [B, HPC, L, HD], bf16, kind="ExternalInput").ap()
    out = nc.dram_tensor("out", [BT, D], fp16, kind="ExternalOutput").ap()

    EXP = mybir.ActivationFunctionType.Exp

    with tile.TileContext(nc) as tc:
        with (
            tc.tile_pool(name="consts", bufs=1) as consts,
            tc.tile_pool(name="persist", bufs=1) as persist,
            tc.tile_pool(name="htiles", bufs=NTC) as htiles,
            tc.tile_pool(name="exps", bufs=20) as exps,
            tc.tile_pool(name="episb", bufs=2) as episb,
            tc.tile_pool(name="scps", bufs=2, space="PSUM") as scps,
            tc.tile_pool(name="avp", bufs=2, space="PSUM") as avp,
            tc.tile_pool(name="mps", bufs=2, space="PSUM") as mps,
        ):
            # ---- PE warm-up: junk matmuls on a memset tile (no gpsimd
            # dependency). ~36 cold matmuls = 3.9us busy trips the HAM
            # activity monitor so the real phase-1 work runs at 2.4 GHz.
            zjunk = consts.tile([128, 128], bf16)
            nc.vector.memset(zjunk, 0.0)
            dum = mps.tile([128, 512], fp32, tag="mm", name="dum")
            for i in range(36):
                nc.tensor.matmul(dum[:, 0:128], zjunk, zjunk,
                                 start=True, stop=True)

            # ---- constants + all hidden chunks: issue every DMA up
            # front. scalar ring: weights (wq first, it gates the first
            # projection). sync/gpsimd rings: hidden g-chunks round-robin;
            # latents lead on gpsimd (needed by attention chunk 0).
            wq_sb = consts.tile([128, NKC * DC], bf16)
            wk_sb = consts.tile([128, NKC * DC], bf16)
            wv_sb = consts.tile([128, NKC * DC], bf16)
            wq_v = wq_sb.rearrange("p (k c) -> p k c", k=NKC)
            wk_v = wk_sb.rearrange("p (k c) -> p k c", k=NKC)
            wv_v = wv_sb.rearrange("p (k c) -> p k c", k=NKC)
            bq_sb = consts.tile([DC, 1], fp32)
            bk_sb = consts.tile([DC, 1], fp32)
            bv1_sb = consts.tile([DC, 1], fp32)
            eb0_sb = consts.tile([128, 1], fp32)
            wo_sb = consts.tile([DC, D], bf16)
            ones64 = consts.tile([1, 64], bf16)
            nc.vector.memset(ones64, 1.0)

            nc.scalar.dma_start(out=wq_sb, in_=wq)
            nc.scalar.dma_start(out=bq_sb, in_=bq)
            nc.scalar.dma_start(out=eb0_sb, in_=ebias0)
            nc.scalar.dma_start(out=wk_sb, in_=wk)
            nc.scalar.dma_start(out=bk_sb, in_=bk)
            nc.scalar.dma_start(out=wv_sb, in_=wv)
            nc.scalar.dma_start(out=bv1_sb, in_=bv1)
            nc.scalar.dma_start(out=wo_sb, in_=wo)

            # persistent activations
            qT_sb = persist.tile([128, BT], bf16)  # [h0|h1 feats, tok]
            kT_sb = persist.tile([128, B * SCHUNKS * 128], bf16)
            v_sb = persist.tile([128, B * HPC * SCHUNKS * 65], bf16)

            def k_off(b):
                return b * SCHUNKS * 128

            def v_off(b, h, c):
                return ((b * HPC + h) * SCHUNKS + c) * 65

            # pad/ones memsets must precede the latent DMAs (WAW on the
            # same regions)
            for b in range(B):
                nc.vector.memset(kT_sb[:, k_off(b) + L:k_off(b) + 128], 0.0)
                for h in range(HPC):
                    nc.vector.memset(
                        v_sb[:, v_off(b, h, 0):v_off(b, h, 0) + 65], 0.0)
            v_view = v_sb.rearrange("p (n c) -> p n c", c=65)
            nc.vector.memset(v_view[:, :, 64:65], 1.0)

            # hidden g-chunks (one DMA each, alternating rings) with the
            # latent K/V on sync right behind g0 (needed by attention
            # chunk 0 at ~6us; gpsimd stays clear for g1/g3/g5/g7)
            hts = []
            for g in range(NTC):
                hts.append(htiles.tile([128, NKC, 512], bf16, tag="ht",
                                       name=f"ht_{g}"))
            # phase-1 chunks g0-g3 split across both rings (full HBM
            # bandwidth each -> ~2.8us/chunk, matching the qkv compute
            # cadence); latents slot in after g0's halves
            for g in range(4):
                nc.sync.dma_start(out=hts[g][:, 0:4, :],
                                  in_=hpre[g][:, 0:4, :])
                nc.gpsimd.dma_start(out=hts[g][:, 4:8, :],
                                    in_=hpre[g][:, 4:8, :])
                if g == 0:
                    for b in range(B):
                        nc.sync.dma_start(
                            out=kT_sb[:, k_off(b):k_off(b) + L], in_=lkT[b])
                    for b in range(B):
                        for h in range(HPC):
                            nc.gpsimd.dma_start(
                                out=v_sb[0:L,
                                         v_off(b, h, 0):v_off(b, h, 0) + HD],
                                in_=lv[b, h])
            for g in range(4, NTC):
                eng = nc.sync if g % 2 == 0 else nc.gpsimd
                eng.dma_start(out=hts[g], in_=hpre[g])

            # identity for PE transposes (gpsimd iota path; only the
            # V-transposes wait on it)
            ident = consts.tile([128, 128], bf16)
            from concourse.masks import make_identity
            make_identity(nc, ident)

            # ---- qkv projection chains for one 512-token chunk ----
            # Three chains of (weight, fn) at single-matmul granularity.
            # Within a chain the "mm"-tag PSUM tile is live, so no other
            # mm-tag user may be traced between its items; the pacing
            # loop only interleaves chunk bodies (sc/avp/ex tags).
            def qkv_chains(g):
                t0g = g * 512
                bb = t0g // T
                c0 = (t0g - bb * T) // 128 + 1
                ht = hts[g]
                hold = {}

                def mk_mm(key, w_v, k):
                    def go():
                        if k == 0:
                            hold[key] = mps.tile([128, 512], fp32, tag="mm",
                                                 name=f"{key}ps_{g}")
                        nc.tensor.matmul(
                            hold[key], w_v[:, k, :], ht[:, k, :],
                            start=(k == 0), stop=(k == NKC - 1))
                    return go

                def q_fin():
                    nc.vector.tensor_scalar_add(
                        qT_sb[:, t0g:t0g + 512], hold["q"], bq_sb)

                def k_fin():
                    koff = k_off(bb) + 128 + (t0g - bb * T)
                    nc.vector.tensor_scalar_add(
                        kT_sb[:, koff:koff + 512], hold["k"], bk_sb)

                def v_fin():
                    vt = episb.tile([128, 512], bf16, tag="vt",
                                    name=f"vt_{g}")
                    nc.vector.tensor_scalar_add(vt, hold["v"], bv1_sb)
                    hold["vt"] = vt

                def t_a():
                    tp = mps.tile([128, 512], bf16, tag="mm",
                                  name=f"tp_{g}")
                    hold["tp"] = tp
                    for j in range(2):
                        nc.tensor.transpose(
                            tp[:, j * 128:(j + 1) * 128],
                            hold["vt"][:, j * 128:(j + 1) * 128], ident)

                def t_b():
                    tp = hold["tp"]
                    for j in range(2, 4):
                        nc.tensor.transpose(
                            tp[:, j * 128:(j + 1) * 128],
                            hold["vt"][:, j * 128:(j + 1) * 128], ident)
                    # v_sb[:, (c0+m, h, d)] = tp[:, (m, h, d)]
                    dst = bass.AP(
                        tensor=v_sb.tensor,
                        offset=v_sb.offset + v_off(bb, 0, c0),
                        ap=[v_sb.ap[0], [65, 4], [SCHUNKS * 65, HPC],
                            [1, HD]])
                    srcv = tp.rearrange("p (m e) -> p m e", m=4)
                    src = bass.AP(
                        tensor=srcv.tensor, offset=srcv.offset,
                        ap=[srcv.ap[0], [128, 4], [64, HPC], [1, HD]])
                    nc.vector.tensor_copy(dst, src)

                qc = [(1.0, mk_mm("q", wq_v, k)) for k in range(NKC)]
                qc.append((0.4, q_fin))
                qc.append((0.0, lambda: done.add(("q", g))))
                kc_ = [(1.0, mk_mm("k", wk_v, k)) for k in range(NKC)]
                kc_.append((0.4, k_fin))
                kc_.append((0.0, lambda: done.add(("k", g))))
                vc = [(1.0, mk_mm("v", wv_v, k)) for k in range(NKC)]
                vc.append((0.4, v_fin))
                vc.append((0.6, t_a))
                vc.append((0.8, t_b))
                vc.append((0.0, lambda: done.add(("v", g))))
                return [qc, kc_, vc]

            # ---- attention helpers ----
            def emit_epi_drain(st, use_scalar=False):
                # use_scalar: on the final epilogue ScalarE is idle, so the
                # PSUM drains run there in parallel with DVE's recip chain,
                # shortening the kernel's serial tail.
                av0, av1, tw = st["av0"], st["av1"], st["tw"]
                oz = episb.tile([128, 512], bf16, tag="oz",
                                name=f"oz_{st['q0']}")
                zh0 = episb.tile([1, 512], fp32, tag="zh0",
                                 name=f"zh0_{st['q0']}")
                zh1 = episb.tile([1, 512], fp32, tag="zh1",
                                 name=f"zh1_{st['q0']}")
                zh0b = episb.tile([1, 512], bf16, tag="zh0b",
                                  name=f"zh0b_{st['q0']}")
                zh1b = episb.tile([1, 512], bf16, tag="zh1b",
                                  name=f"zh1b_{st['q0']}")
                nc.vector.tensor_copy(zh0[:, :tw], av0[64:65, :tw])
                nc.vector.tensor_copy(zh1[:, :tw], av1[64:65, :tw])
                if use_scalar:
                    nc.scalar.copy(oz[0:64, :tw], av0[0:64, :tw])
                    nc.scalar.copy(oz[64:128, :tw], av1[0:64, :tw])
                else:
                    nc.vector.tensor_copy(oz[0:64, :tw], av0[0:64, :tw])
                    nc.vector.tensor_copy(oz[64:128, :tw], av1[0:64, :tw])
                nc.vector.reciprocal_approx_fast(out=zh0[:, :tw],
                                                 in_=zh0[:, :tw])
                nc.vector.reciprocal_approx_fast(out=zh1[:, :tw],
                                                 in_=zh1[:, :tw])
                nc.vector.tensor_copy(zh0b[:, :tw], zh0[:, :tw])
                nc.vector.tensor_copy(zh1b[:, :tw], zh1[:, :tw])
                st["oz"], st["zhb"] = oz, (zh0b, zh1b)

            def emit_epi_zb(st):
                tw = st["tw"]
                zh0b, zh1b = st["zhb"]
                zb = mps.tile([128, 512], fp32, tag="mm",
                              name=f"zb_{st['q0']}")
                # broadcast 1/Z over each head's 64 rows via two K=1
                # col-tiled matmuls (concurrent; replaces the former
                # zr2-row-DMA roundtrip on the epilogue critical path)
                nc.tensor.matmul(zb[0:64, :tw], ones64, zh0b[:, :tw],
                                 start=True, stop=True)
                nc.tensor.matmul(zb[64:128, :tw], ones64, zh1b[:, :tw],
                                 start=True, stop=True)
                ot = episb.tile([128, 512], bf16, tag="ot",
                                name=f"ot_{st['q0']}")
                nc.vector.tensor_mul(ot[:, :tw], st["oz"][:, :tw],
                                     zb[:, :tw])
                st["ot"] = ot

            def mk_epi_out(st, j, f, final=False):
                # one out-projection matmul + cast per side item, so the
                # PE<->DVE ping-pong through the mm-tag PSUM ring always
                # has attention work traced between its steps
                def go():
                    if j >= st["tw"] // 128:
                        return
                    ot, q0 = st["ot"], st["q0"]
                    r0 = q0 + j * 128
                    if f == 0:
                        st[f"osb{j}"] = episb.tile(
                            [128, 1024], fp16, tag="osb", bufs=4,
                            name=f"osb_{q0}_{j}")
                    osb = st[f"osb{j}"]
                    # final epilogue: the av banks are drained, so half
                    # the out-proj PSUM tiles rotate through the avp ring
                    # instead - doubles the slack in the PE<->DVE cast
                    # ping-pong that otherwise serializes the tail
                    pool, tg = ((avp, "avp") if final and f == 0
                                else (mps, "mm"))
                    op = pool.tile([128, 512], fp32, tag=tg,
                                   name=f"op_{q0}_{j}_{f}")
                    nc.tensor.matmul(
                        op, ot[:, j * 128:(j + 1) * 128],
                        wo_sb[:, f * 512:(f + 1) * 512],
                        start=True, stop=True)
                    # final epilogue: ScalarE is idle, split the PSUM
                    # drains across both engines to halve the tail
                    if final and f == 1:
                        nc.scalar.copy(osb[:, f * 512:(f + 1) * 512], op)
                    else:
                        nc.vector.tensor_copy(
                            osb[:, f * 512:(f + 1) * 512], op)
                    if f == 1:
                        if final:
                            eng = (nc.sync, nc.gpsimd, nc.scalar,
                                   nc.sync)[j]
                        else:
                            eng = (nc.sync if (q0 // 512 + j) % 2 == 0
                                   else nc.gpsimd)
                        eng.dma_start(out=out[r0:r0 + 128, :], in_=osb)
                return go

            def emit_av(st, c, ex):
                b, tw = st["b"], st["tw"]
                stt, sp = c == 0, c == SCHUNKS - 1
                for h, av in ((0, st["av0"]), (1, st["av1"])):
                    vo = v_off(b, h, c)
                    eh = ex[:, h * tw:(h + 1) * tw]
                    nc.tensor.matmul(
                        av[:, :tw], v_sb[:, vo:vo + 65], eh,
                        start=stt, stop=sp)

            def make_st(b, q0, tw=TB):
                return {
                    "b": b, "q0": q0, "tw": tw,
                    "av0": avp.tile([65, 512], fp32, tag="avp",
                                    name=f"av0_{q0}"),
                    "av1": avp.tile([65, 512], fp32, tag="avp",
                                    name=f"av1_{q0}"),
                }

            # schedule: 8 t-blocks, batch-0 then batch-1
            schedule = ([(0, q, TB) for q in range(0, T, TB)] +
                        [(1, T + q, TB) for q in range(0, T, TB)])

            def produce(bi, c):
                # scores + exp for (block, chunk); the ex tile is stashed
                # in exq for the (possibly much later) AV consumer.
                b, q0, tw = schedule[bi]
                sc = scps.tile([128, 1024], fp32, tag="sc",
                               name=f"sc_{b}_{q0}_{c}")
                kc = k_off(b) + c * 128
                nc.tensor.matmul(
                    sc[:, 0:tw],
                    kT_sb[0:64, kc:kc + 128],
                    qT_sb[0:64, q0:q0 + tw],
                    start=True, stop=True)
                nc.tensor.matmul(
                    sc[:, tw:2 * tw],
                    kT_sb[64:128, kc:kc + 128],
                    qT_sb[64:128, q0:q0 + tw],
                    start=True, stop=True)
                ex = exps.tile([128, 1024], bf16, tag="ex",
                               name=f"ex_{b}_{q0}_{c}")
                nc.scalar.activation(
                    ex[:, 0:2 * tw], sc[:, 0:2 * tw], EXP,
                    bias=(eb0_sb if c == 0 else 0.0), scale=1.0)
                exq[(bi, c)] = ex

            def queue_epilogue(st, bi, side, final=False):
                emit_av(st, SCHUNKS - 1, exq.pop((bi, SCHUNKS - 1)))
                emit_epi_drain(st, use_scalar=final)
                # queued as one chain; two no-op slots let the drain
                # chain finish before zb consumes zr2

                def mk_zb(s):
                    def go():
                        emit_epi_zb(s)
                    return go
                noop = lambda: None
                from collections import deque as _dq
                items = [(1.0, noop), (1.0, noop), (1.2, mk_zb(st))]
                for j in range(4):
                    for f in range(2):
                        items.append((1.2, mk_epi_out(st, j, f, final)))
                side.appendleft(_dq(items))

            pnext = {}  # block -> next unproduced chunk

            def try_lookahead(nb):
                # opportunistically produce the next block's scores+exp so
                # ScalarE stays fed while this block consumes its stash
                if nb >= len(schedule):
                    return False
                cn = pnext.get(nb, 0)
                if cn >= SCHUNKS or len(exq) >= 18:
                    return False
                bb, qq0, _ = schedule[nb]
                need = [("q", qq0 // 512)]
                if cn >= 1:
                    need.append(("k", 4 * bb + (cn - 1) // 4))
                if not all(m in done for m in need):
                    return False
                produce(nb, cn)
                pnext[nb] = cn + 1
                return True

            def chunk_body(st, bi, c):
                if pnext.get(bi, 0) > c:
                    try_lookahead(bi + 1)  # (bi, c) already stashed
                else:
                    produce(bi, c)
                    pnext[bi] = c + 1
                if len(exq) < 10:
                    # lead attrited (markers were blocked); rebuild it
                    try_lookahead(bi) or try_lookahead(bi + 1)
                if c >= 1:
                    emit_av(st, c - 1, exq.pop((bi, c - 1)))

            # ---- phase 1: qkv for batch 0, interleaved with the first
            # t-block's attention chunks. The first attention chunk (the
            # latent prefix) only needs q, so it runs right after the q
            # projection to get ScalarE's table load out of the way. ----
            from collections import deque
            side = deque()  # deque of chains
            done = set()  # traced q/k/v completion markers, per g
            exq = {}  # (block, chunk) -> stashed ex tile
            qc0, kc0, vc0 = qkv_chains(0)
            for w, f in qc0:
                f()
            st0 = make_st(0, 0)
            chunk_body(st0, 0, 0)
            for w, f in kc0 + vc0:
                f()
            for g in (1, 2, 3):
                flat = [it for ch in qkv_chains(g) for it in ch]
                base = 4 * (g - 1)
                bounds = [0, 8, 16, 24, len(flat)]
                for i in range(4):
                    chunk_body(st0, 0, base + 1 + i)
                    for w, f in flat[bounds[i]:bounds[i + 1]]:
                        f()
                    # bank next-block exp work: fills ScalarE during
                    # the projection-heavy opening (markers gate safety)
                    try_lookahead(1)
                    try_lookahead(1)
            for c in range(13, SCHUNKS):
                chunk_body(st0, 0, c)
                try_lookahead(1)
            sts = {0: st0}
            queue_epilogue(st0, 0, side)
            for g in range(NTC // 2, NTC):
                for ch in qkv_chains(g):
                    side.append(deque(ch))

            # ---- phase 2: remaining t-blocks with paced side work
            # (epilogues, batch-1 qkv). Budget ~1.75 matmul-equivalents of
            # side work per attention chunk: ScalarE's exp (1.07us/chunk)
            # stays the critical path with the PE just underneath it. ----
            nblocks = len(schedule)
            cur = deque()  # partially-drained head chain

            def drain_one():
                nonlocal cur
                if not cur:
                    if not side:
                        return None
                    cur = side.popleft()
                w, f = cur.popleft()
                f()
                return w

            def drain_until(*marks):
                # Trace-order safety: a consumer traced before its data
                # producer would read stale memory, so force the producer
                # chains through before tracing the consumer.
                for m in marks:
                    while m not in done:
                        assert drain_one() is not None, f"missing {m}"

            for bi, (b, q0, tw) in enumerate(schedule):
                if bi == 0:
                    continue
                if b == 1:
                    drain_until(("q", q0 // 512))
                st = make_st(b, q0, tw)
                last = bi == nblocks - 1
                bank = 0.0
                for c in range(SCHUNKS):
                    if b == 1 and c >= 1:
                        drain_until(("k", 4 + (c - 1) // 4),
                                    ("v", 4 + max(0, c - 2) // 4))
                    chunk_body(st, bi, c)
                    if c >= 1:
                        bank += 3.4 if last else 1.75
                        popped = 0
                        while bank > 0 and popped < (12 if last else 3):
                            w = drain_one()
                            if w is None:
                                break
                            bank -= w
                            popped += 1
                if b == 1:
                    drain_until(("v", 7))
                queue_epilogue(st, bi, side, final=last)
            # flush remaining side work (the last epilogue)
            while cur:
                cur.popleft()[1]()
            while side:
                ch = side.popleft()
                while ch:
                    ch.popleft()[1]()

    nc.compile()
    return nc


def _get_nc():
    if "nc" not in _cache:
        _cache["nc"] = _build_nc()
    return _cache["nc"]


def _prep_inputs(hidden_states, decoder_latent, Wq, bq, Wk, bk, Wv, bv, Wo):
    """Build the 8 per-core input maps (host-side sharding/layout)."""
    x2 = np.asarray(hidden_states, np.float32).reshape(BT, D)
    # hpre[g, p, k, t] = x2[g*512 + t, k*128 + p]
    hpre = np.ascontiguousarray(
        x2.reshape(NTC, 512, NKC, 128).transpose(0, 3, 2, 1)).astype(BF16)

    def wprep(w):
        # [D, DC] -> [128, NKC*DC] with row p = concat_k w[k*128+p, :]
        return np.ascontiguousarray(
            w.reshape(NKC, 128, DC).transpose(1, 0, 2).reshape(128, NKC * DC)
        ).astype(BF16)

    lk = decoder_latent[..., :HD]  # [B, H, L, HD]
    lvf = decoder_latent[..., HD:]
    eb0 = np.full((128, 1), PAD_BIAS, np.float32)
    eb0[:L] = 0.0
    in_maps = []
    for c in range(NCORES):
        cols = slice(c * DC, (c + 1) * DC)
        h0, h1 = HPC * c, HPC * c + 1
        lkT_c = np.stack([
            np.concatenate([lk[b, h0].T, lk[b, h1].T], axis=0)
            for b in range(B)])  # [B, 128, L]
        in_maps.append({
            "hpre": hpre,
            "wq": wprep(Wq[:, cols] * SCALE),
            "wk": wprep(Wk[:, cols]),
            "wv": wprep(Wv[:, cols]),
            "bq": (bq[cols] * SCALE).astype(np.float32).reshape(DC, 1),
            "bk": bk[cols].astype(np.float32).reshape(DC, 1),
            "bv1": bv[cols].astype(np.float32).reshape(DC, 1),
            "ebias0": eb0,
            "wo": Wo[cols, :].astype(BF16),
            "lkT": lkT_c.astype(BF16),
            "lv": lvf[:, h0:h1 + 1].astype(BF16),
        })
    return in_maps


def _run(inputs, trace=False):
    from concourse.bass_utils import run_bass_kernel_spmd

    nc = _get_nc()
    in_maps = _prep_inputs(
        inputs["hidden_states"], inputs["decoder_latent"],
        inputs["Wq"], inputs["bq"], inputs["Wk"], inputs["bk"],
        inputs["Wv"], inputs["bv"], inputs["Wo"])
    res = run_bass_kernel_spmd(nc, in_maps, core_ids=list(range(NCORES)),
                               trace=trace)
    acc = np.zeros((BT, D), np.float64)
    for r in res.results:
        acc += r["out"].astype(np.float64)
    out = (acc + inputs["bo"].astype(np.float64)).astype(np.float32)
    return out.reshape(B, T, D), res


def _reference_fallback(hidden_states, decoder_latent, attention_mask,
                        Wq, bq, Wk, bk, Wv, bv, Wo, bo):
    """Exact numpy path, used only when attention_mask is non-zero (the
    problem spec fills it with zeros; the device kernel specializes on
    that)."""
    x = hidden_states.astype(np.float64)
    q = (x @ Wq + bq) * SCALE
    k = x @ Wk + bk
    v = x @ Wv + bv

    def heads(a):
        return a.reshape(B, T, H, HD).transpose(0, 2, 1, 3)

    q, k, v = heads(q), heads(k), heads(v)
    lk = decoder_latent[..., :HD].astype(np.float64)
    lv = decoder_latent[..., HD:].astype(np.float64)
    k = np.concatenate([lk, k], axis=2)
    v = np.concatenate([lv, v], axis=2)
    s = np.einsum("bhtd,bhsd->bhts", q, k) + attention_mask.astype(np.float64)
    s -= s.max(axis=-1, keepdims=True)
    p = np.exp(s)
    p /= p.sum(axis=-1, keepdims=True)
    o = np.einsum("bhts,bhsd->bhtd", p, v)
    o = o.transpose(0, 2, 1, 3).reshape(B, T, D)
    return (o @ Wo + bo).astype(np.float32)


def kernel(**inputs):
    inputs = {k: np.asarray(v) for k, v in inputs.items()}
    if np.any(inputs["attention_mask"]):
        return _reference_fallback(**inputs)
    out, _ = _run(inputs)
    return out
